# revision 1
# baseline (speedup 1.0000x reference)
"""Varlen causal attention (flash_attn_varlen semantics) on 8 Trainium2 cores.

Sharding: 16 heads across 8 cores (2 heads/core, Ulysses-style head shard,
identity comms). Each core runs the same SPMD Bass program on its head slice.

Per head: blocked attention over 128-row q blocks. For each q block only the
k blocks inside the (causal x segment) mask are computed -- the block structure
is specialized on the host from cu_seqlens at trace time. S = Q^T K runs in
float32r; P = exp(S * scale) in bf16 (logits are O(5), so no max subtraction
is needed); a ones-column appended to V yields the softmax denominator from
the same PV matmul.
"""

import numpy as np

L = 4096
H = 16
D = 128
N_CORES = 8
H_PER_CORE = H // N_CORES
SCALE = 1.0 / float(np.sqrt(D))
QB = 128  # q/k block size


def _seg_starts(cu: np.ndarray) -> np.ndarray:
    """Per-token segment start, exactly mirroring the reference searchsorted."""
    tok = np.arange(L)
    seg = np.searchsorted(cu[1:-1], tok, side="right")
    starts = np.concatenate([[0], cu[1:-1]])
    return starts[seg]


def _chunk_sizes(w: int) -> list:
    """Split w (multiple of 128) into matmul chunks <=512, avoiding <256
    trailing chunks (float32r runs at 1/4 rate below 256 free dim)."""
    sizes = [512] * (w // 512)
    rem = w % 512
    if rem:
        sizes.append(rem)
    if len(sizes) > 1 and sizes[-1] < 256:
        sizes[-2:] = [sizes[-2] - 128, sizes[-1] + 128]
    return sizes


def _build(cu: np.ndarray):
    import concourse.mybir as mybir
    import concourse.tile as tile
    from concourse import bacc
    from concourse.masks import make_identity

    f32 = mybir.dt.float32
    f32r = mybir.dt.float32r
    bf16 = mybir.dt.bfloat16
    AF = mybir.ActivationFunctionType

    seg_start = _seg_starts(cu)
    n_qb = L // QB

    nc = bacc.Bacc("TRN2", target_bir_lowering=False, debug=False,
                   num_devices=N_CORES)
    q_d = nc.dram_tensor("q", [L, H_PER_CORE, D], f32, kind="ExternalInput")
    k_d = nc.dram_tensor("k", [L, H_PER_CORE, D], f32, kind="ExternalInput")
    v_d = nc.dram_tensor("v", [L, H_PER_CORE, D], f32, kind="ExternalInput")
    o_d = nc.dram_tensor("out", [L, H_PER_CORE, D], f32, kind="ExternalOutput")

    with tile.TileContext(nc) as tc:
        with (
            tc.tile_pool(name="consts", bufs=1) as consts,
            tc.tile_pool(name="big", bufs=2) as big,
            tc.tile_pool(name="io", bufs=6) as io,
            tc.tile_pool(name="psb", bufs=3) as psb,
            tc.tile_pool(name="t_ps", bufs=2, space="PSUM") as tr_ps_pool,
            tc.tile_pool(name="s_ps", bufs=2, space="PSUM") as s_ps_pool,
            tc.tile_pool(name="o_ps", bufs=2, space="PSUM") as o_ps_pool,
        ):
            ident = consts.tile([128, 128], f32)
            make_identity(nc, ident[:])
            ident_bf = consts.tile([128, 128], bf16)
            nc.vector.tensor_copy(ident_bf[:], ident[:])

            for h in range(H_PER_CORE):
                # ---- prep: load + transpose Q,K; load + cast V (+ones col)
                qt_sb = big.tile([128, L], f32r, tag="qt")
                kt_sb = big.tile([128, L], f32r, tag="kt")
                v_sb = big.tile([128, n_qb, 132], bf16, tag="v")
                for t in range(n_qb):
                    r = slice(t * QB, (t + 1) * QB)
                    q_t = io.tile([128, D], f32, tag="q_in")
                    nc.sync.dma_start(q_t[:], q_d[r, h, :])
                    tp = tr_ps_pool.tile([128, 128], f32, tag="t")
                    nc.tensor.transpose(tp[:], q_t[:], ident[:])
                    nc.vector.tensor_copy(qt_sb[:, r], tp[:])

                    k_t = io.tile([128, D], f32, tag="k_in")
                    nc.sync.dma_start(k_t[:], k_d[r, h, :])
                    tp2 = tr_ps_pool.tile([128, 128], f32, tag="t")
                    nc.tensor.transpose(tp2[:], k_t[:], ident[:])
                    nc.vector.tensor_copy(kt_sb[:, r], tp2[:])

                    v_t = io.tile([128, D], f32, tag="v_in")
                    nc.sync.dma_start(v_t[:], v_d[r, h, :])
                    nc.vector.memset(v_sb[:, t, 0:1], 1.0)
                    nc.vector.tensor_copy(v_sb[:, t, 1:129], v_t[:])

                # ---- main: per q block
                for i in range(n_qb):
                    q0 = i * QB
                    k_lo_b = int(seg_start[q0]) // QB
                    k_lo = k_lo_b * QB
                    w = (i + 1) * QB - k_lo

                    p_sb = psb.tile([128, L], bf16, tag="p")

                    # S = (Q^T)^T K^T, chunked; P = exp(S * scale)
                    c0 = 0
                    for cw in _chunk_sizes(w):
                        s_ps = s_ps_pool.tile([128, 512], f32)
                        nc.tensor.matmul(
                            s_ps[:, :cw],
                            qt_sb[:, q0:q0 + QB],
                            kt_sb[:, k_lo + c0:k_lo + c0 + cw],
                            start=True, stop=True,
                        )
                        nc.scalar.activation(p_sb[:, c0:c0 + cw], s_ps[:, :cw],
                                             AF.Exp, scale=SCALE)
                        c0 += cw

                    # segment-boundary masking: rows whose segment starts at
                    # b > k_lo must drop columns [k_lo, b). Zeroing those
                    # columns for all rows >= b works because later segments
                    # need a superset zeroed. Partition offsets must be
                    # 32-aligned, so row-conditional zeroing goes through
                    # affine_select (predicate on the partition index).
                    for b in sorted(set(int(s) for s in seg_start[q0:q0 + QB])):
                        ncols = b - k_lo
                        if ncols <= 0:
                            continue
                        rb = b - q0
                        if rb <= 0:
                            nc.vector.memset(p_sb[:, 0:ncols], 0.0)
                        else:
                            # keep row p iff p < rb  <=>  (rb-1-p) >= 0
                            nc.gpsimd.affine_select(
                                out=p_sb[:, 0:ncols], in_=p_sb[:, 0:ncols],
                                compare_op=mybir.AluOpType.is_ge, fill=0.0,
                                base=rb - 1, pattern=[[0, ncols]],
                                channel_multiplier=-1,
                            )

                    # causal triangle on the diagonal block
                    nc.gpsimd.affine_select(
                        out=p_sb[:, w - QB:w], in_=p_sb[:, w - QB:w],
                        compare_op=mybir.AluOpType.is_ge, fill=0.0,
                        base=0, pattern=[[-1, QB]], channel_multiplier=1,
                    )

                    # O[:, 0] = denom, O[:, 1:129] = P @ V
                    o_ps = o_ps_pool.tile([128, 129], f32)
                    for j in range(k_lo_b, i + 1):
                        pt_sb = io.tile([128, 128], bf16, tag="pt")
                        nc.scalar.dma_start(
                            pt_sb[:], p_sb[:, (j - k_lo_b) * QB:(j - k_lo_b + 1) * QB],
                            transpose=True)
                        nc.tensor.matmul(o_ps[:], pt_sb[:], v_sb[:, j, 0:129],
                                         start=(j == k_lo_b), stop=(j == i))

                    recip = io.tile([128, 1], f32, tag="recip")
                    nc.vector.reciprocal(recip[:], o_ps[:, 0:1])
                    o_sb = io.tile([128, D], f32, tag="o_out")
                    nc.vector.tensor_scalar_mul(o_sb[:], o_ps[:, 1:129], recip[:])
                    nc.sync.dma_start(o_d[i * QB:(i + 1) * QB, h, :], o_sb[:])

    nc.compile()
    return nc


def _run(query, key, value, cu_seqlens, trace=False, **spmd_kwargs):
    from concourse import bass_utils

    query = np.ascontiguousarray(np.asarray(query, dtype=np.float32))
    key = np.ascontiguousarray(np.asarray(key, dtype=np.float32))
    value = np.ascontiguousarray(np.asarray(value, dtype=np.float32))
    cu = np.asarray(cu_seqlens, dtype=np.int64)

    nc = _build(cu)
    in_maps = []
    for c in range(N_CORES):
        hs = slice(c * H_PER_CORE, (c + 1) * H_PER_CORE)
        in_maps.append({
            "q": np.ascontiguousarray(query[:, hs, :]),
            "k": np.ascontiguousarray(key[:, hs, :]),
            "v": np.ascontiguousarray(value[:, hs, :]),
        })
    res = bass_utils.run_bass_kernel_spmd(nc, in_maps, list(range(N_CORES)),
                                          trace=trace, **spmd_kwargs)
    out = np.empty((L, H, D), dtype=np.float32)
    for c in range(N_CORES):
        out[:, c * H_PER_CORE:(c + 1) * H_PER_CORE, :] = res.results[c]["out"]
    return out, res


def kernel(query, key, value, cu_seqlens):
    out, _ = _run(query, key, value, cu_seqlens)
    return out



# revision 6
# speedup vs baseline: 8.2130x; 8.2130x over previous
"""Varlen causal attention (flash_attn_varlen semantics) on 8 Trainium2 cores.

Sharding: 16 heads across 8 cores (2 heads/core, Ulysses-style head shard,
identity comms). Each core runs the same SPMD Bass program on its head slice.

Per head the kernel computes S^T = (K^T)^T @ (Q^T) directly in the [k, q]
orientation, so P^T = exp(S^T * scale) lands in SBUF already transposed for
the PV matmul (lhsT = P^T block, rhs = V) -- no P transpose DMAs at all.
All matmuls run in bf16 (1 cycle/row on the PE). A ones-column prepended to
V yields the softmax denominator from the same PV accumulation. The
(causal x segment) block structure is specialized on the host from
cu_seqlens at trace time; segment-boundary and causal masking is done with
affine_select on the Pool engine over the exp'd P^T strips.
"""

import numpy as np

L = 4096
H = 16
D = 128
N_CORES = 8
H_PER_CORE = H // N_CORES
SCALE = 1.0 / float(np.sqrt(D))
QB = 128          # q/k block size
NB = L // QB      # 32 blocks
P_CAP = 188       # p ring capacity in 128-col blocks (47KB/partition bf16)


def _plan(cu: np.ndarray):
    """Host-side schedule from cu_seqlens, mirroring the reference
    searchsorted semantics exactly."""
    tok = np.arange(L)
    seg = np.searchsorted(cu[1:-1], tok, side="right")
    starts = np.concatenate([[0], cu[1:-1]])
    seg_start = starts[seg]

    # block-aligned causal/segment structure
    klo = [int(seg_start[i * QB]) // QB for i in range(NB)]          # per q block
    e = []                                                           # per k block
    for j in range(NB):
        mx = j
        for i in range(j, NB):
            if klo[i] <= j:
                mx = i
        e.append(mx)

    bnds = sorted(set(int(b) for b in cu[1:-1] if 0 < int(b) < L))
    bnd_in = {j: [b for b in bnds if j * QB < b < (j + 1) * QB] for j in range(NB)}
    # first boundary inside q block i (local offset), if any: queries >= that
    # boundary must not see ANY key from earlier k blocks
    bnd_first = {i: (bnd_in[i][0] - i * QB) for i in range(NB) if bnd_in[i]}

    # q tiles: greedy ranges [a, b) whose live P^T blocks fit the ring
    def live(a, b):
        s = 0
        for j in range(b):
            if e[j] >= a:
                s += min(e[j], b - 1) - max(j, a) + 1
        return s

    tiles = []
    a = 0
    while a < NB:
        b = a + 1
        while b < NB and live(a, b + 1) <= P_CAP:
            b += 1
        tiles.append((a, b))
        a = b
    return klo, e, bnd_in, bnd_first, tiles


def _build(cu: np.ndarray):
    import concourse.mybir as mybir
    import concourse.tile as tile
    from concourse import bacc
    from concourse.masks import make_identity

    f32 = mybir.dt.float32
    bf16 = mybir.dt.bfloat16
    AF = mybir.ActivationFunctionType
    GE = mybir.AluOpType.is_ge

    klo, e, bnd_in, bnd_first, tiles = _plan(cu)

    nc = bacc.Bacc("TRN2", target_bir_lowering=False, debug=False,
                   num_devices=N_CORES)
    q_d = nc.dram_tensor("q", [L, H_PER_CORE, D], f32, kind="ExternalInput")
    k_d = nc.dram_tensor("k", [L, H_PER_CORE, D], f32, kind="ExternalInput")
    v_d = nc.dram_tensor("v", [L, H_PER_CORE, D], f32, kind="ExternalInput")
    o_d = nc.dram_tensor("out", [L, H_PER_CORE, D], f32, kind="ExternalOutput")

    with tile.TileContext(nc) as tc:
        with (
            tc.tile_pool(name="consts", bufs=1) as consts,
            tc.tile_pool(name="pring", bufs=1) as pring,
            tc.tile_pool(name="big", bufs=2) as big,
            tc.tile_pool(name="io", bufs=3) as io,
            tc.tile_pool(name="s_ps", bufs=2, space="PSUM") as s_pool,
            tc.tile_pool(name="o_ps", bufs=2, space="PSUM") as o_pool,
            tc.tile_pool(name="t_ps", bufs=2, space="PSUM") as t_pool,
        ):
            ident_bf = consts.tile([128, 128], bf16)
            make_identity(nc, ident_bf[:])
            p_all = pring.tile([128, P_CAP, QB], bf16)

            # ---- prefetch all input tiles for both heads (sync queue) ----
            qin, kin, vin = {}, {}, {}
            for h in range(H_PER_CORE):
                for g in range(8):
                    t0 = g * 4 * QB
                    src = slice(t0, t0 + 4 * QB)
                    q_t = io.tile([128, 4, D], f32, tag="q_in")
                    nc.sync.dma_start(
                        q_t[:], q_d[src, h, :].rearrange("(u p) d -> p u d", p=128))
                    k_t = io.tile([128, 4, D], f32, tag="k_in")
                    nc.sync.dma_start(
                        k_t[:], k_d[src, h, :].rearrange("(u p) d -> p u d", p=128))
                    v_t = io.tile([128, 4, D], f32, tag="v_in")
                    nc.sync.dma_start(
                        v_t[:], v_d[src, h, :].rearrange("(u p) d -> p u d", p=128))
                    qin[h, g], kin[h, g], vin[h, g] = q_t, k_t, v_t

            for h in range(H_PER_CORE):
                # ---- prep: transpose Q,K to [d, token]; cast V (+ones col) ----
                qt = big.tile([128, NB, QB], bf16, tag="qt")
                kt = big.tile([128, NB, QB], bf16, tag="kt")
                vs = big.tile([128, NB, 132], bf16, tag="vs")
                nc.vector.memset(vs[:, :, 0:1], 1.0)
                for g in range(8):
                    qbf = io.tile([128, 4, D], bf16, tag="qbf")
                    nc.gpsimd.tensor_copy(qbf[:], qin[h, g][:])
                    tp = t_pool.tile([128, 4, 128], bf16, tag="tr")
                    for u in range(4):
                        nc.tensor.transpose(tp[:, u, :], qbf[:, u, :], ident_bf[:])
                    nc.vector.tensor_copy(qt[:, g * 4:(g + 1) * 4, :], tp[:])

                    kbf = io.tile([128, 4, D], bf16, tag="kbf")
                    nc.gpsimd.tensor_copy(kbf[:], kin[h, g][:])
                    tp2 = t_pool.tile([128, 4, 128], bf16, tag="tr")
                    for u in range(4):
                        nc.tensor.transpose(tp2[:, u, :], kbf[:, u, :], ident_bf[:])
                    nc.vector.tensor_copy(kt[:, g * 4:(g + 1) * 4, :], tp2[:])

                    nc.vector.tensor_copy(vs[:, g * 4:(g + 1) * 4, 1:129],
                                          vin[h, g][:])

                for (a, b) in tiles:
                    # ---- strip parts present in this q tile ----
                    parts = {}
                    off = 0
                    for j in range(b):
                        if e[j] < a:
                            continue
                        qsb = max(j, a)
                        qeb = min(e[j], b - 1)
                        parts[j] = (qsb, qeb, off)
                        off += qeb - qsb + 1

                    # ---- QK phase: S^T strips -> exp -> masks ----
                    for j, (qsb, qeb, off) in parts.items():
                        n = qeb - qsb + 1
                        for c0 in range(0, n, 8):
                            nblk = min(8, n - c0)
                            sp = s_pool.tile([128, 8, 128], f32, tag="s")
                            for m0 in range(0, nblk, 4):
                                mm = min(4, nblk - m0)
                                nc.tensor.matmul(
                                    sp[:, m0:m0 + mm, :],
                                    kt[:, j, :],
                                    qt[:, qsb + c0 + m0:qsb + c0 + m0 + mm, :],
                                    start=True, stop=True)
                            nc.scalar.activation(
                                p_all[:, off + c0:off + c0 + nblk, :],
                                sp[:, 0:nblk, :], AF.Exp, scale=SCALE)

                        # causal triangle on the diagonal block (keep q >= k)
                        if qsb == j:
                            nc.gpsimd.affine_select(
                                out=p_all[:, off, :], in_=p_all[:, off, :],
                                compare_op=GE, fill=0.0, base=0,
                                pattern=[[1, QB]], channel_multiplier=-1)
                        # segment boundaries inside k block j: zero k rows
                        # p < r for q columns >= b (earlier-segment keys
                        # invisible to later-segment queries)
                        for bnd in bnd_in[j]:
                            r = bnd - j * QB
                            if qsb == j:
                                nc.gpsimd.affine_select(
                                    out=p_all[:, off, r:QB],
                                    in_=p_all[:, off, r:QB],
                                    compare_op=GE, fill=0.0, base=-r,
                                    pattern=[[0, QB - r]], channel_multiplier=1)
                                if n > 1:
                                    nc.gpsimd.affine_select(
                                        out=p_all[:, off + 1:off + n, :],
                                        in_=p_all[:, off + 1:off + n, :],
                                        compare_op=GE, fill=0.0, base=-r,
                                        pattern=[[0, n - 1], [0, QB]],
                                        channel_multiplier=1)
                            else:
                                nc.gpsimd.affine_select(
                                    out=p_all[:, off:off + n, :],
                                    in_=p_all[:, off:off + n, :],
                                    compare_op=GE, fill=0.0, base=-r,
                                    pattern=[[0, n], [0, QB]],
                                    channel_multiplier=1)

                        # q blocks later than j that contain a segment start:
                        # queries past that boundary see nothing of k block j
                        for i in range(max(qsb, j + 1), qeb + 1):
                            r = bnd_first.get(i)
                            if r is not None:
                                nc.gpsimd.memset(
                                    p_all[:, off + (i - qsb), r:QB], 0.0)

                    # ---- PV phase: O[:,0]=denom, O[:,1:129]=P@V ----
                    op = None
                    for i in range(a, b):
                        u3 = (i - a) % 3
                        if u3 == 0:
                            op = o_pool.tile([128, 3, 129], f32, tag="o")
                        jlo = klo[i]
                        for j in range(jlo, i + 1):
                            qsb, qeb, off = parts[j]
                            idx = off + (i - qsb)
                            nc.tensor.matmul(
                                op[:, u3, :], p_all[:, idx, :], vs[:, j, 0:129],
                                start=(j == jlo), stop=(j == i))
                        if u3 == 2 or i == b - 1:
                            nn = u3 + 1
                            i0 = i - u3
                            rc = io.tile([128, 3, 1], f32, tag="rc")
                            nc.vector.reciprocal(rc[:, 0:nn, :], op[:, 0:nn, 0:1])
                            o3 = io.tile([128, 3, D], f32, tag="o3")
                            for u in range(nn):
                                nc.vector.tensor_scalar_mul(
                                    o3[:, u, :], op[:, u, 1:129], rc[:, u, 0:1])
                            nc.sync.dma_start(
                                o_d[i0 * QB:(i0 + nn) * QB, h, :]
                                .rearrange("(u p) d -> p u d", p=128),
                                o3[:, 0:nn, :])

    nc.compile()
    return nc


def _run(query, key, value, cu_seqlens, trace=False, **spmd_kwargs):
    from concourse import bass_utils

    query = np.ascontiguousarray(np.asarray(query, dtype=np.float32))
    key = np.ascontiguousarray(np.asarray(key, dtype=np.float32))
    value = np.ascontiguousarray(np.asarray(value, dtype=np.float32))
    cu = np.asarray(cu_seqlens, dtype=np.int64)

    nc = _build(cu)
    in_maps = []
    for c in range(N_CORES):
        hs = slice(c * H_PER_CORE, (c + 1) * H_PER_CORE)
        in_maps.append({
            "q": np.ascontiguousarray(query[:, hs, :]),
            "k": np.ascontiguousarray(key[:, hs, :]),
            "v": np.ascontiguousarray(value[:, hs, :]),
        })
    res = bass_utils.run_bass_kernel_spmd(nc, in_maps, list(range(N_CORES)),
                                          trace=trace, **spmd_kwargs)
    out = np.empty((L, H, D), dtype=np.float32)
    for c in range(N_CORES):
        out[:, c * H_PER_CORE:(c + 1) * H_PER_CORE, :] = res.results[c]["out"]
    return out, res


def kernel(query, key, value, cu_seqlens):
    out, _ = _run(query, key, value, cu_seqlens)
    return out


# revision 10
# speedup vs baseline: 10.0761x; 1.2269x over previous
"""Varlen causal attention (flash_attn_varlen semantics) on 8 Trainium2 cores.

Sharding: 16 heads across 8 cores (2 heads/core, Ulysses-style head shard,
identity comms). Each core runs the same SPMD Bass program on its head slice.

Per head the kernel computes S^T = (K^T)^T @ (Q^T) directly in the [k, q]
orientation, so P^T = exp(S^T * scale) lands in SBUF already transposed for
the PV matmul (lhsT = P^T block, rhs = V) -- no P transpose DMAs at all.
All matmuls run in bf16 (1 cycle/row on the PE). A ones-column prepended to
V yields the softmax denominator from the same PV accumulation. The
(causal x segment) block structure is specialized on the host from
cu_seqlens at trace time; segment-boundary and causal masking is done with
affine_select on the Pool engine over the exp'd P^T strips.
"""

import numpy as np

L = 4096
H = 16
D = 128
N_CORES = 8
H_PER_CORE = H // N_CORES
SCALE = 1.0 / float(np.sqrt(D))
QB = 128          # q/k block size
NB = L // QB      # 32 blocks
P_CAP = 188       # p ring capacity in 128-col blocks (47KB/partition bf16)


def _plan(cu: np.ndarray):
    """Host-side schedule from cu_seqlens, mirroring the reference
    searchsorted semantics exactly."""
    tok = np.arange(L)
    seg = np.searchsorted(cu[1:-1], tok, side="right")
    starts = np.concatenate([[0], cu[1:-1]])
    seg_start = starts[seg]

    # block-aligned causal/segment structure
    klo = [int(seg_start[i * QB]) // QB for i in range(NB)]          # per q block
    e = []                                                           # per k block
    for j in range(NB):
        mx = j
        for i in range(j, NB):
            if klo[i] <= j:
                mx = i
        e.append(mx)

    bnds = sorted(set(int(b) for b in cu[1:-1] if 0 < int(b) < L))
    bnd_in = {j: [b for b in bnds if j * QB < b < (j + 1) * QB] for j in range(NB)}
    # first boundary inside q block i (local offset), if any: queries >= that
    # boundary must not see ANY key from earlier k blocks
    bnd_first = {i: (bnd_in[i][0] - i * QB) for i in range(NB) if bnd_in[i]}

    # q tiles: greedy ranges [a, b) whose live P^T blocks fit the ring
    def live(a, b):
        s = 0
        for j in range(b):
            if e[j] >= a:
                s += min(e[j], b - 1) - max(j, a) + 1
        return s

    tiles = []
    a = 0
    while a < NB:
        b = a + 1
        while b < NB and live(a, b + 1) <= P_CAP:
            b += 1
        tiles.append((a, b))
        a = b
    return klo, e, bnd_in, bnd_first, tiles


def _build(cu: np.ndarray):
    import concourse.mybir as mybir
    import concourse.tile as tile
    from concourse import bacc
    from concourse.masks import make_identity

    f32 = mybir.dt.float32
    bf16 = mybir.dt.bfloat16
    AF = mybir.ActivationFunctionType
    GE = mybir.AluOpType.is_ge

    klo, e, bnd_in, bnd_first, tiles = _plan(cu)

    nc = bacc.Bacc("TRN2", target_bir_lowering=False, debug=False,
                   num_devices=N_CORES)
    q_d = nc.dram_tensor("q", [L, H_PER_CORE, D], f32, kind="ExternalInput")
    k_d = nc.dram_tensor("k", [L, H_PER_CORE, D], f32, kind="ExternalInput")
    v_d = nc.dram_tensor("v", [L, H_PER_CORE, D], f32, kind="ExternalInput")
    o_d = nc.dram_tensor("out", [L, H_PER_CORE, D], f32, kind="ExternalOutput")

    with tile.TileContext(nc) as tc:
        with (
            tc.tile_pool(name="consts", bufs=1) as consts,
            tc.tile_pool(name="pring", bufs=1) as pring,
            tc.tile_pool(name="big", bufs=2) as big,
            tc.tile_pool(name="io", bufs=3) as io,
            tc.tile_pool(name="s_ps", bufs=2, space="PSUM") as s_pool,
            tc.tile_pool(name="o_ps", bufs=2, space="PSUM") as o_pool,
            tc.tile_pool(name="t_ps", bufs=2, space="PSUM") as t_pool,
        ):
            ident_bf = consts.tile([128, 128], bf16)
            make_identity(nc, ident_bf[:])
            p_all = pring.tile([128, P_CAP, QB], bf16)

            # ---- prefetch all input tiles for both heads (sync queue) ----
            qin, kin, vin = {}, {}, {}
            for h in range(H_PER_CORE):
                for g in range(8):
                    t0 = g * 4 * QB
                    src = slice(t0, t0 + 4 * QB)
                    q_t = io.tile([128, 4, D], f32, tag="q_in")
                    nc.sync.dma_start(
                        q_t[:], q_d[src, h, :].rearrange("(u p) d -> p u d", p=128))
                    k_t = io.tile([128, 4, D], f32, tag="k_in")
                    nc.sync.dma_start(
                        k_t[:], k_d[src, h, :].rearrange("(u p) d -> p u d", p=128))
                    v_t = io.tile([128, 4, D], f32, tag="v_in")
                    nc.sync.dma_start(
                        v_t[:], v_d[src, h, :].rearrange("(u p) d -> p u d", p=128))
                    qin[h, g], kin[h, g], vin[h, g] = q_t, k_t, v_t

            state = {h: {} for h in range(H_PER_CORE)}

            def cast_qk_thunks(h):
                """Per-group Q/K bf16 casts: Q on DVE, K on Act."""
                st = state[h]
                st["qbf"] = big.tile([128, NB, QB], bf16, tag="qbf", name="qbf")
                st["kbf"] = big.tile([128, NB, QB], bf16, tag="kbf", name="kbf")
                def mk(g):
                    def emit():
                        sl = slice(g * 4, (g + 1) * 4)
                        nc.vector.tensor_copy(st["qbf"][:, sl, :], qin[h, g][:])
                        nc.scalar.copy(st["kbf"][:, sl, :], kin[h, g][:])
                    return emit
                return [mk(g) for g in range(8)]

            def prep_v(h):
                """V bf16 cast with ones column (DVE, during QK idle)."""
                st = state[h]
                vs = st["vs"] = big.tile([128, NB, 132], bf16, tag="vs", name="vs")
                nc.vector.memset(vs[:, :, 0:1], 1.0)
                for g in range(8):
                    nc.vector.tensor_copy(vs[:, g * 4:(g + 1) * 4, 1:129],
                                          vin[h, g][:])

            def prep_tr(h):
                """PE-transpose Q,K into [d, token] layout."""
                st = state[h]
                qt = st["qt"] = big.tile([128, NB, QB], bf16, tag="qt", name="qt")
                kt = st["kt"] = big.tile([128, NB, QB], bf16, tag="kt", name="kt")
                for g in range(8):
                    tp = t_pool.tile([128, 4, 128], bf16, tag="tr")
                    for u in range(4):
                        nc.tensor.transpose(tp[:, u, :], st["qbf"][:, g * 4 + u, :],
                                            ident_bf[:])
                    nc.vector.tensor_copy(qt[:, g * 4:(g + 1) * 4, :], tp[:])
                    tp2 = t_pool.tile([128, 4, 128], bf16, tag="tr")
                    for u in range(4):
                        nc.tensor.transpose(tp2[:, u, :], st["kbf"][:, g * 4 + u, :],
                                            ident_bf[:])
                    nc.vector.tensor_copy(kt[:, g * 4:(g + 1) * 4, :], tp2[:])

            def qk_phase(h, a, b, parts):
                """S^T strips -> exp -> masks, chunked across strip ends."""
                qt, kt = state[h]["qt"], state[h]["kt"]

                def emit_masks(j, qsb, qeb, off):
                    n = qeb - qsb + 1
                    if qsb == j:
                        nc.gpsimd.affine_select(
                            out=p_all[:, off, :], in_=p_all[:, off, :],
                            compare_op=GE, fill=0.0, base=0,
                            pattern=[[1, QB]], channel_multiplier=-1)
                    for bnd in bnd_in[j]:
                        r = bnd - j * QB
                        if qsb == j:
                            nc.gpsimd.affine_select(
                                out=p_all[:, off, r:QB],
                                in_=p_all[:, off, r:QB],
                                compare_op=GE, fill=0.0, base=-r,
                                pattern=[[0, QB - r]], channel_multiplier=1)
                            if n > 1:
                                nc.gpsimd.affine_select(
                                    out=p_all[:, off + 1:off + n, :],
                                    in_=p_all[:, off + 1:off + n, :],
                                    compare_op=GE, fill=0.0, base=-r,
                                    pattern=[[0, n - 1], [0, QB]],
                                    channel_multiplier=1)
                        else:
                            nc.gpsimd.affine_select(
                                out=p_all[:, off:off + n, :],
                                in_=p_all[:, off:off + n, :],
                                compare_op=GE, fill=0.0, base=-r,
                                pattern=[[0, n], [0, QB]],
                                channel_multiplier=1)
                    for i in range(max(qsb, j + 1), qeb + 1):
                        r = bnd_first.get(i)
                        if r is not None:
                            nc.gpsimd.memset(
                                p_all[:, off + (i - qsb), r:QB], 0.0)

                # flat (strip j, q block) sequence; p index == position
                seq = []
                for j, (qsb, qeb, off) in parts.items():
                    for i in range(qsb, qeb + 1):
                        seq.append((j, i, i == qeb))
                pos = 0
                while pos < len(seq):
                    nblk = min(8, len(seq) - pos)
                    sp = s_pool.tile([128, 8, 128], f32, tag="s")
                    # matmul runs: same strip, within one 4-block psum bank
                    r0 = 0
                    while r0 < nblk:
                        j0 = seq[pos + r0][0]
                        r1 = r0 + 1
                        lim = 4 if r0 < 4 else 8
                        while (r1 < nblk and r1 < lim and
                               seq[pos + r1][0] == j0):
                            r1 += 1
                        qs0 = seq[pos + r0][1]
                        nc.tensor.matmul(
                            sp[:, r0:r1, :], kt[:, j0, :],
                            qt[:, qs0:qs0 + (r1 - r0), :],
                            start=True, stop=True)
                        r0 = r1
                    nc.scalar.activation(
                        p_all[:, pos:pos + nblk, :], sp[:, 0:nblk, :],
                        AF.Exp, scale=SCALE)
                    # strips fully exp'd by this chunk -> masks
                    for t in range(nblk):
                        j, i, is_end = seq[pos + t]
                        if is_end:
                            emit_masks(j, *parts[j])
                    pos += nblk

            def pv_phase(h, a, b, parts, interleave=()):
                qt, kt, vs = state[h]["qt"], state[h]["kt"], state[h]["vs"]
                pending = list(interleave)
                op = None
                for i in range(a, b):
                    u3 = (i - a) % 3
                    if u3 == 0:
                        op = o_pool.tile([128, 3, 129], f32, tag="o")
                    jlo = klo[i]
                    for j in range(jlo, i + 1):
                        qsb, qeb, off = parts[j]
                        idx = off + (i - qsb)
                        nc.tensor.matmul(
                            op[:, u3, :], p_all[:, idx, :], vs[:, j, 0:129],
                            start=(j == jlo), stop=(j == i))
                    if u3 == 2 or i == b - 1:
                        nn = u3 + 1
                        i0 = i - u3
                        rc = io.tile([128, 3, 1], f32, tag="rc")
                        nc.vector.reciprocal(rc[:, 0:nn, :], op[:, 0:nn, 0:1])
                        o3 = io.tile([128, 3, D], f32, tag="o3")
                        nc.vector.tensor_mul(
                            o3[:, 0:nn, :], op[:, 0:nn, 1:129],
                            rc[:, 0:nn, :].broadcast_to([128, nn, D]))
                        nc.sync.dma_start(
                            o_d[i0 * QB:(i0 + nn) * QB, h, :]
                            .rearrange("(u p) d -> p u d", p=128),
                            o3[:, 0:nn, :])
                        if pending:
                            pending.pop(0)()
                for fn in pending:
                    fn()

            def mk_parts(a, b):
                parts = {}
                off = 0
                for j in range(b):
                    if e[j] < a:
                        continue
                    qsb = max(j, a)
                    qeb = min(e[j], b - 1)
                    parts[j] = (qsb, qeb, off)
                    off += qeb - qsb + 1
                return parts

            # pipeline: head h+1 Q/K casts interleave with head h PV so the
            # DVE/Act queues stay just ahead of the PE
            for fn in cast_qk_thunks(0):
                fn()
            prep_tr(0)
            for h in range(H_PER_CORE):
                all_tiles = [(ti, ab) for ti, ab in enumerate(tiles)]
                for ti, (a, b) in all_tiles:
                    parts = mk_parts(a, b)
                    if ti == 0:
                        prep_v(h)
                    qk_phase(h, a, b, parts)
                    last = ti == len(tiles) - 1
                    if last and h + 1 < H_PER_CORE:
                        thunks = cast_qk_thunks(h + 1)
                        pv_phase(h, a, b, parts, interleave=thunks)
                        prep_tr(h + 1)
                    else:
                        pv_phase(h, a, b, parts)

    nc.compile()
    return nc


def _run(query, key, value, cu_seqlens, trace=False, **spmd_kwargs):
    from concourse import bass_utils

    query = np.ascontiguousarray(np.asarray(query, dtype=np.float32))
    key = np.ascontiguousarray(np.asarray(key, dtype=np.float32))
    value = np.ascontiguousarray(np.asarray(value, dtype=np.float32))
    cu = np.asarray(cu_seqlens, dtype=np.int64)

    nc = _build(cu)
    in_maps = []
    for c in range(N_CORES):
        hs = slice(c * H_PER_CORE, (c + 1) * H_PER_CORE)
        in_maps.append({
            "q": np.ascontiguousarray(query[:, hs, :]),
            "k": np.ascontiguousarray(key[:, hs, :]),
            "v": np.ascontiguousarray(value[:, hs, :]),
        })
    res = bass_utils.run_bass_kernel_spmd(nc, in_maps, list(range(N_CORES)),
                                          trace=trace, **spmd_kwargs)
    out = np.empty((L, H, D), dtype=np.float32)
    for c in range(N_CORES):
        out[:, c * H_PER_CORE:(c + 1) * H_PER_CORE, :] = res.results[c]["out"]
    return out, res


def kernel(query, key, value, cu_seqlens):
    out, _ = _run(query, key, value, cu_seqlens)
    return out
